# revision 41
# baseline (speedup 1.0000x reference)
"""Trainium2 Bass kernel for AttentiveTransformer:
   out = sparsemax(GBN(feat @ W.T) * priors)

Data-parallel over 8 NeuronCores: batch 131072 rows -> 8 shards of 16384.
Per core: 32 superchunks x 512 rows; each superchunk = 4 GBN chunks (VBS=128).

v2 pipeline (per superchunk):
  - batched DMA for feat/priors/out
  - feat -> PE transpose -> featT (f32r, rounded by the ACT copy)
  - fp32r matmul per d-slice: x_s = WT_s.T @ featT -> PSUM [128d, 512r]
  - xs = ACT copy PSUM->SBUF; ONE segmented bn_stats per slice
    ([128,4,128] -> [128,4,6]) gives per-chunk even/odd stats
  - gpsimd merges stats -> scale/shift per (slice, chunk)
  - normalize (x*scale+shift -> fp16) split across ACT/DVE/gpsimd
  - PE fp16 transpose back -> ztp [128r, 512d]; z = ztp * priors (fp16)
  - top-16 per row: 4x max8 on 128-col segments -> 32 cands, max8 ->
    top8, match_replace suppress, max8 -> next8
  - tau via batched [128,4,16] prefix math (gpsimd), closed form
  - ACT final: out = relu(z - tau)
"""
import sys

sys.path.insert(0, "/opt/trn_rl_repo")

import numpy as np
from contextlib import ExitStack

import concourse.bass as bass
import concourse.bacc as bacc
import concourse.tile as tile
from concourse.tile import add_dep_helper
from concourse import mybir
from concourse.bass_utils import run_bass_kernel_spmd

f32 = mybir.dt.float32
f32r = mybir.dt.float32r
f16 = mybir.dt.float16
AF = mybir.ActivationFunctionType
OP = mybir.AluOpType

N_CORES = 8
B, IN, D = 131072, 128, 512
ROWS = B // N_CORES          # 16384 rows per core
SC_ROWS = 512                # superchunk rows (4 GBN chunks)
N_SC = ROWS // SC_ROWS       # 32
VBS = 128
EPS = 1e-5
NEG = -1.0e9


def build_nc():
    nc = bacc.Bacc(None, target_bir_lowering=False)

    priors = nc.dram_tensor("priors", [ROWS, D], f32, kind="ExternalInput")
    feat = nc.dram_tensor("processed_feat", [ROWS, IN], f32, kind="ExternalInput")
    Wd = nc.dram_tensor("W", [D, IN], f32, kind="ExternalInput")
    gam = nc.dram_tensor("gamma", [D], f32, kind="ExternalInput")
    bet = nc.dram_tensor("beta", [D], f32, kind="ExternalInput")
    out = nc.dram_tensor("out", [ROWS, D], f32, kind="ExternalOutput")

    with tile.TileContext(nc) as tc, ExitStack() as ctx:
        singles = ctx.enter_context(tc.tile_pool(name="singles", bufs=1))
        ft_pool = ctx.enter_context(tc.tile_pool(name="ft", bufs=3))
        xs_pool = ctx.enter_context(tc.tile_pool(name="xs", bufs=10))
        xn_pool = ctx.enter_context(tc.tile_pool(name="xn", bufs=12))
        p_pool = ctx.enter_context(tc.tile_pool(name="p", bufs=4))
        z_pool = ctx.enter_context(tc.tile_pool(name="z", bufs=10))
        o_pool = ctx.enter_context(tc.tile_pool(name="o", bufs=4))
        st_pool = ctx.enter_context(tc.tile_pool(name="st", bufs=4))
        sm_pool = ctx.enter_context(tc.tile_pool(name="sm", bufs=24))
        ps_ft = ctx.enter_context(tc.tile_pool(name="psft", bufs=2, space="PSUM"))
        ps_x = ctx.enter_context(tc.tile_pool(name="psx", bufs=3, space="PSUM"))
        ps_zt = ctx.enter_context(tc.tile_pool(name="pszt", bufs=3, space="PSUM"))

        # ---------- one-time constants ----------
        ident = singles.tile([128, 128], f32)
        nc.gpsimd.iota(ident, [[1, 128]], base=0, channel_multiplier=-1,
                       allow_small_or_imprecise_dtypes=True)
        nc.vector.tensor_scalar(ident, ident, 0.0, None, OP.is_equal)
        ident_h = singles.tile([128, 128], f16)
        nc.vector.tensor_copy(ident_h, ident)

        # WT [128k, 512d] resident, f32r (ACT copy performs the rounding)
        WT = singles.tile([128, D], f32r)
        wtp = ps_ft.tile([128, D], f32, tag="ftp")
        for s in range(4):
            wtile = ft_pool.tile([128, 128], f32, tag="wtile")
            nc.sync.dma_start(out=wtile, in_=Wd[s * 128:(s + 1) * 128, :])
            nc.tensor.transpose(wtp[:, s * 128:(s + 1) * 128], wtile, ident)
        nc.scalar.copy(WT, wtp)

        # gamma/beta broadcast [128, 4slice, 4chunk]
        gamma44 = singles.tile([128, 4, 4], f32)
        beta44 = singles.tile([128, 4, 4], f32)
        gamma4 = singles.tile([128, 4], f32)
        beta4 = singles.tile([128, 4], f32)
        gr = gam.rearrange("(s p) -> s p", p=128)
        br = bet.rearrange("(s p) -> s p", p=128)
        for s4 in range(4):
            nc.sync.dma_start(out=gamma4[:, s4:s4 + 1],
                              in_=gr[s4].rearrange("(p o) -> p o", o=1))
            nc.sync.dma_start(out=beta4[:, s4:s4 + 1],
                              in_=br[s4].rearrange("(p o) -> p o", o=1))
        for c4 in range(4):
            nc.vector.tensor_copy(gamma44[:, :, c4], gamma4)
            nc.vector.tensor_copy(beta44[:, :, c4], beta4)

        eps_t = singles.tile([128, 1], f32)
        nc.vector.memset(eps_t, EPS)
        c32 = singles.tile([128, 4, 4], f32)
        nc.vector.memset(c32, 32.0)
        cnh = singles.tile([128, 4, 4], f32)
        nc.vector.memset(cnh, -0.5)

        rho16 = singles.tile([128, 16], f32)
        nc.gpsimd.iota(rho16, [[1, 16]], base=1, channel_multiplier=0,
                       allow_small_or_imprecise_dtypes=True)
        invrho = singles.tile([128, 16], f32)
        nc.vector.reciprocal(invrho, rho16)
        rho16q = singles.tile([128, 4, 16], f16)
        invrhoq = singles.tile([128, 4, 16], f32)
        for jj in range(4):
            nc.vector.tensor_copy(rho16q[:, jj], rho16)
            nc.vector.tensor_copy(invrhoq[:, jj], invrho)

        fe_r = feat.rearrange("(n c p) k -> n p c k", p=128, c=4)
        pr_r = priors.rearrange("(n c p) d -> n p c d", p=128, c=4)
        out_r = out.rearrange("(n c p) d -> n p c d", p=128, c=4)

        # ---------- main loop (chunk-interleaved software pipeline) ----------
        def emit_load(sc):
            f4 = ft_pool.tile([128, 4, 128], f32, tag="f4")
            nc.sync.dma_start(out=f4, in_=fe_r[sc])
            ftp = ps_ft.tile([128, SC_ROWS], f32, tag="ftp")
            for q in range(4):
                nc.tensor.transpose(ftp[:, q * 128:(q + 1) * 128], f4[:, q],
                                    ident)
            featT = ft_pool.tile([128, SC_ROWS], f32r, tag="featT")
            nc.scalar.copy(featT, ftp)
            p4 = p_pool.tile([128, 4, D], f32)
            nc.sync.dma_start(out=p4, in_=pr_r[sc])
            stats = st_pool.tile([128, 4, 2, 6], f16)
            return dict(featT=featT, p4=p4, stats=stats, xss=[], bn_insts=[])

        def emit_gemm(st, s):
            xp = ps_x.tile([128, SC_ROWS], f32)
            nc.tensor.matmul(xp, WT[:, s * 128:(s + 1) * 128], st["featT"])
            st.setdefault("xps", []).append(xp)

        def emit_copy_bn(st, s):
            # xs chunk-contiguous fp16; bn_stats reads each chunk-pair with an
            # interleaved AP (i-major, chunk-inner) so the even/odd stat sets
            # are exactly the per-chunk stats.  InstBNStats is built directly
            # because bass's bn_stats() treats 3D inputs as segmented.
            xs = xs_pool.tile([128, 4, VBS], f16)
            nc.scalar.copy(xs, st["xps"][s])
            for j in range(2):
                in3d = xs[:, 2 * j:2 * j + 2, :].transpose([0, 2, 1])
                bi = nc.vector.add_instruction(
                    mybir.InstBNStats(
                        name=nc.vector.bass.get_next_instruction_name(),
                        ins=[nc.vector.lower_ap(in3d)],
                        outs=[nc.vector.lower_ap(st["stats"][:, s, j])]))
                st["bn_insts"].append(bi)
            st["xss"].append(xs)

        def emit_stats(st):
            # stats[p, s, j, (g h)]: g = chunk parity within pair j, h = field
            # (count, mean, M2); chunk c = 2j+g. Per-chunk mean/var direct.
            stats6 = st["stats"].rearrange("p s j (g h) -> p s j g h", g=2)
            mv = stats6[:, :, :, :, 1]
            M2v = stats6[:, :, :, :, 2]
            bn_insts = st["bn_insts"]
            sd = sm_pool.tile([128, 4, 4], f32, tag="sd")
            isd = sm_pool.tile([128, 4, 4], f32, tag="isd")
            sscale = sm_pool.tile([128, 4, 4], f32, tag="sscale")
            tsh = sm_pool.tile([128, 4, 4], f32, tag="tsh")
            tshift = sm_pool.tile([128, 4, 4], f32, tag="tshift")
            i1 = nc.scalar.activation(sd, M2v, AF.Sqrt, bias=eps_t,
                                      scale=1.0 / VBS)
            add_dep_helper(i1.ins, bn_insts[-1].ins, sync=True, reason="bn")
            for bi in bn_insts:
                add_dep_helper(i1.ins, bi.ins, sync=True, reason="stats raw")
            nc.vector.reciprocal(isd, sd)
            nc.gpsimd.tensor_tensor(sscale, isd, gamma44, OP.mult)
            i2 = nc.vector.tensor_tensor(tsh, mv, sscale, OP.mult)
            for bi in bn_insts:
                add_dep_helper(i2.ins, bi.ins, sync=True, reason="stats raw")
            nc.gpsimd.tensor_tensor(tshift, beta44, tsh, OP.subtract)
            st["sscale"] = sscale
            st["tshift"] = tshift

        def emit_chunk_pre(st, c):
            sscale = st["sscale"]
            tshift = st["tshift"]
            xnc = xn_pool.tile([128, 4, VBS], f16, tag="xnc")
            for s in range(4):
                sc_ap = sscale[:, s, c:c + 1]
                sh_ap = tshift[:, s, c:c + 1]
                src = st["xss"][s][:, c]
                dst = xnc[:, s]
                if s < 3 or c < 2:
                    nc.scalar.activation(out=dst, in_=src, func=AF.Identity,
                                         bias=sh_ap, scale=sc_ap)
                else:
                    nc.vector.tensor_scalar(dst, src, sc_ap, sh_ap,
                                            OP.mult, OP.add)
            j = c // 2
            if c % 2 == 0:
                st["ztp2"] = ps_zt.tile([128, 2, D], f16, name="ztp2")
            ztp2 = st["ztp2"]
            for s in range(4):
                nc.tensor.transpose(ztp2[:, c % 2, s * 128:(s + 1) * 128],
                                    xnc[:, s], ident_h)
            if c % 2 == 1:
                z2 = z_pool.tile([128, 2, D], f16, tag="z")
                nc.vector.tensor_tensor(z2, ztp2, st["p4"][:, 2 * j:2 * j + 2],
                                        OP.mult)
                t16 = st["t16q"]
                for cc in (2 * j, 2 * j + 1):
                    z = z2[:, cc % 2]
                    st["z"][cc] = z
                    l1 = sm_pool.tile([128, 16], f16, tag="l1")
                    for h in range(2):
                        nc.vector.max(out=l1[:, h * 8:(h + 1) * 8],
                                      in_=z[:, h * 256:(h + 1) * 256])
                    nc.vector.max(out=t16[:, cc, 0:8], in_=l1)
                    l1m = sm_pool.tile([128, 16], f16, tag="l1m")
                    nc.vector.match_replace(l1m, t16[:, cc, 0:8], l1, NEG)
                    nc.vector.max(out=t16[:, cc, 8:16], in_=l1m)

        def emit_sc_tail(st, sci):
            t16 = st["t16q"]
            # one flat 64-wide cumsum, then per-chunk fixup: for chunk c,
            # cs_c[i] = fy[16c+i] - (fy[16c-1] + 1), with fy[-1] := -1.
            fy = sm_pool.tile([128, 64], f32, tag="fy")
            t16ff = t16.rearrange("p c i -> p (c i)")
            nc.vector.tensor_tensor_scan(fy, t16ff, t16ff,
                                         -1.0, OP.add, OP.bypass)
            offsp = sm_pool.tile([128, 4, 1], f32, tag="offsp")
            nc.gpsimd.memset(offsp[:, 0:1], 0.0)
            nc.vector.tensor_scalar(offsp[:, 1:4],
                                    fy.rearrange("p (c i) -> p c i", i=16)
                                    [:, 0:3, 15:16], 1.0, None, OP.add)
            cs = sm_pool.tile([128, 4, 16], f32, tag="cs")
            nc.vector.tensor_tensor(
                cs, fy.rearrange("p (c i) -> p c i", i=16),
                offsp.broadcast_to([128, 4, 16]),
                OP.subtract)
            rz = sm_pool.tile([128, 4, 16], f32, tag="rz")
            nc.gpsimd.tensor_tensor(rz, t16, rho16q, OP.mult)
            cond = sm_pool.tile([128, 4, 17], f32, tag="cond")
            nc.gpsimd.memset(cond[:, :, 16:17], 0.0)
            nc.vector.tensor_tensor(cond[:, :, 0:16], rz, cs, OP.is_gt)
            dcn = sm_pool.tile([128, 4, 16], f32, tag="dcn")
            nc.gpsimd.tensor_tensor(dcn, cond[:, :, 1:17], cond[:, :, 0:16],
                                    OP.subtract)
            tauj = sm_pool.tile([128, 4, 16], f32, tag="tauj")
            nc.gpsimd.tensor_tensor(tauj, cs, invrhoq, OP.mult)
            scr = sm_pool.tile([128, 4, 16], f32, tag="scr")
            negtau = sm_pool.tile([128, 4], f32, tag="negtau")
            nc.gpsimd.tensor_tensor(scr, tauj, dcn, OP.mult)
            nc.vector.tensor_reduce(out=negtau, in_=scr,
                                    axis=mybir.AxisListType.X, op=OP.add)
            for c in range(4):
                nc.scalar.activation(st["o4"][:, c], st["z"][c], AF.Relu,
                                     bias=negtau[:, c:c + 1], scale=1.0)
            nc.sync.dma_start(out=out_r[sci], in_=st["o4"])

        pend = None
        for sc in range(N_SC):
            st = emit_load(sc)
            if pend is not None:
                pend["o4"] = o_pool.tile([128, 4, D], f32, name="o4", tag="o4")
                pend["z"] = {}
                pend["t16q"] = sm_pool.tile([128, 4, 16], f16, name="t16q",
                                            tag="t16q")
            for i in range(4):
                if pend is not None:
                    emit_chunk_pre(pend, i)
                emit_gemm(st, i)
                emit_copy_bn(st, i)
            if pend is not None:
                emit_sc_tail(pend, sc - 1)
            emit_stats(st)
            pend = st
        pend["o4"] = o_pool.tile([128, 4, D], f32, name="o4", tag="o4")
        pend["z"] = {}
        pend["t16q"] = sm_pool.tile([128, 4, 16], f16, name="t16q", tag="t16q")
        for i in range(4):
            emit_chunk_pre(pend, i)
        emit_sc_tail(pend, N_SC - 1)

    nc.compile()
    return nc


_NC_CACHE = None


def kernel(**inputs) -> np.ndarray:
    global _NC_CACHE
    if _NC_CACHE is None:
        _NC_CACHE = build_nc()
    nc = _NC_CACHE

    priors = np.ascontiguousarray(inputs["priors"], dtype=np.float32)
    feat = np.ascontiguousarray(inputs["processed_feat"], dtype=np.float32)
    W = np.ascontiguousarray(inputs["W"], dtype=np.float32)
    gamma = np.ascontiguousarray(inputs["gamma"], dtype=np.float32)
    beta = np.ascontiguousarray(inputs["beta"], dtype=np.float32)

    in_maps = []
    for i in range(N_CORES):
        sl = slice(i * ROWS, (i + 1) * ROWS)
        in_maps.append({
            "priors": priors[sl],
            "processed_feat": feat[sl],
            "W": W,
            "gamma": gamma,
            "beta": beta,
        })
    res = run_bass_kernel_spmd(nc, in_maps, core_ids=list(range(N_CORES)))
    return np.concatenate([r["out"] for r in res.results], axis=0)


if __name__ == "__main__":
    rng = np.random.default_rng(0)
    inputs = {
        "priors": rng.random((B, D), dtype=np.float32),
        "processed_feat": rng.standard_normal((B, IN), dtype=np.float32),
        "W": (rng.standard_normal((D, IN), dtype=np.float32) * 0.1),
        "gamma": np.ones(D, dtype=np.float32),
        "beta": np.zeros(D, dtype=np.float32),
    }
    out = kernel(**inputs)
    print("out", out.shape, out.dtype, float(out.sum()))


# revision 42
# speedup vs baseline: 1.0327x; 1.0327x over previous
"""Trainium2 Bass kernel for AttentiveTransformer:
   out = sparsemax(GBN(feat @ W.T) * priors)

Data-parallel over 8 NeuronCores: batch 131072 rows -> 8 shards of 16384.
Per core: 32 superchunks x 512 rows; each superchunk = 4 GBN chunks (VBS=128).

v2 pipeline (per superchunk):
  - batched DMA for feat/priors/out
  - feat -> PE transpose -> featT (f32r, rounded by the ACT copy)
  - fp32r matmul per d-slice: x_s = WT_s.T @ featT -> PSUM [128d, 512r]
  - xs = ACT copy PSUM->SBUF; ONE segmented bn_stats per slice
    ([128,4,128] -> [128,4,6]) gives per-chunk even/odd stats
  - gpsimd merges stats -> scale/shift per (slice, chunk)
  - normalize (x*scale+shift -> fp16) split across ACT/DVE/gpsimd
  - PE fp16 transpose back -> ztp [128r, 512d]; z = ztp * priors (fp16)
  - top-16 per row: 4x max8 on 128-col segments -> 32 cands, max8 ->
    top8, match_replace suppress, max8 -> next8
  - tau via batched [128,4,16] prefix math (gpsimd), closed form
  - ACT final: out = relu(z - tau)
"""
import sys

sys.path.insert(0, "/opt/trn_rl_repo")

import numpy as np
from contextlib import ExitStack

import concourse.bass as bass
import concourse.bacc as bacc
import concourse.tile as tile
from concourse.tile import add_dep_helper
from concourse import mybir
from concourse.bass_utils import run_bass_kernel_spmd

f32 = mybir.dt.float32
f32r = mybir.dt.float32r
f16 = mybir.dt.float16
AF = mybir.ActivationFunctionType
OP = mybir.AluOpType

N_CORES = 8
B, IN, D = 131072, 128, 512
ROWS = B // N_CORES          # 16384 rows per core
SC_ROWS = 512                # superchunk rows (4 GBN chunks)
N_SC = ROWS // SC_ROWS       # 32
VBS = 128
EPS = 1e-5
NEG = -1.0e9


def build_nc():
    nc = bacc.Bacc(None, target_bir_lowering=False)

    priors = nc.dram_tensor("priors", [ROWS, D], f32, kind="ExternalInput")
    feat = nc.dram_tensor("processed_feat", [ROWS, IN], f32, kind="ExternalInput")
    Wd = nc.dram_tensor("W", [D, IN], f32, kind="ExternalInput")
    gam = nc.dram_tensor("gamma", [D], f32, kind="ExternalInput")
    bet = nc.dram_tensor("beta", [D], f32, kind="ExternalInput")
    out = nc.dram_tensor("out", [ROWS, D], f32, kind="ExternalOutput")

    with tile.TileContext(nc) as tc, ExitStack() as ctx:
        singles = ctx.enter_context(tc.tile_pool(name="singles", bufs=1))
        ft_pool = ctx.enter_context(tc.tile_pool(name="ft", bufs=3))
        xs_pool = ctx.enter_context(tc.tile_pool(name="xs", bufs=10))
        xn_pool = ctx.enter_context(tc.tile_pool(name="xn", bufs=12))
        p_pool = ctx.enter_context(tc.tile_pool(name="p", bufs=4))
        z_pool = ctx.enter_context(tc.tile_pool(name="z", bufs=10))
        o_pool = ctx.enter_context(tc.tile_pool(name="o", bufs=4))
        st_pool = ctx.enter_context(tc.tile_pool(name="st", bufs=4))
        sm_pool = ctx.enter_context(tc.tile_pool(name="sm", bufs=24))
        ps_ft = ctx.enter_context(tc.tile_pool(name="psft", bufs=2, space="PSUM"))
        ps_x = ctx.enter_context(tc.tile_pool(name="psx", bufs=3, space="PSUM"))
        ps_zt = ctx.enter_context(tc.tile_pool(name="pszt", bufs=3, space="PSUM"))

        # ---------- one-time constants ----------
        ident = singles.tile([128, 128], f32)
        nc.gpsimd.iota(ident, [[1, 128]], base=0, channel_multiplier=-1,
                       allow_small_or_imprecise_dtypes=True)
        nc.vector.tensor_scalar(ident, ident, 0.0, None, OP.is_equal)
        ident_h = singles.tile([128, 128], f16)
        nc.vector.tensor_copy(ident_h, ident)

        # WT [128k, 512d] resident, f32r (ACT copy performs the rounding)
        WT = singles.tile([128, D], f32r)
        wtp = ps_ft.tile([128, D], f32, tag="ftp")
        for s in range(4):
            wtile = ft_pool.tile([128, 128], f32, tag="wtile")
            nc.sync.dma_start(out=wtile, in_=Wd[s * 128:(s + 1) * 128, :])
            nc.tensor.transpose(wtp[:, s * 128:(s + 1) * 128], wtile, ident)
        nc.scalar.copy(WT, wtp)

        # gamma/beta broadcast [128, 4slice, 4chunk]
        gamma44 = singles.tile([128, 4, 4], f32)
        beta44 = singles.tile([128, 4, 4], f32)
        gamma4 = singles.tile([128, 4], f32)
        beta4 = singles.tile([128, 4], f32)
        gr = gam.rearrange("(s p) -> s p", p=128)
        br = bet.rearrange("(s p) -> s p", p=128)
        for s4 in range(4):
            nc.sync.dma_start(out=gamma4[:, s4:s4 + 1],
                              in_=gr[s4].rearrange("(p o) -> p o", o=1))
            nc.sync.dma_start(out=beta4[:, s4:s4 + 1],
                              in_=br[s4].rearrange("(p o) -> p o", o=1))
        for c4 in range(4):
            nc.vector.tensor_copy(gamma44[:, :, c4], gamma4)
            nc.vector.tensor_copy(beta44[:, :, c4], beta4)

        eps_t = singles.tile([128, 1], f32)
        nc.vector.memset(eps_t, EPS)
        c32 = singles.tile([128, 4, 4], f32)
        nc.vector.memset(c32, 32.0)
        cnh = singles.tile([128, 4, 4], f32)
        nc.vector.memset(cnh, -0.5)

        rho16 = singles.tile([128, 16], f32)
        nc.gpsimd.iota(rho16, [[1, 16]], base=1, channel_multiplier=0,
                       allow_small_or_imprecise_dtypes=True)
        invrho = singles.tile([128, 16], f32)
        nc.vector.reciprocal(invrho, rho16)
        rho16q = singles.tile([128, 4, 16], f16)
        invrhoq = singles.tile([128, 4, 16], f32)
        for jj in range(4):
            nc.vector.tensor_copy(rho16q[:, jj], rho16)
            nc.vector.tensor_copy(invrhoq[:, jj], invrho)

        fe_r = feat.rearrange("(n c p) k -> n p c k", p=128, c=4)
        pr_r = priors.rearrange("(n c p) d -> n p c d", p=128, c=4)
        out_r = out.rearrange("(n c p) d -> n p c d", p=128, c=4)

        # ---------- main loop (chunk-interleaved software pipeline) ----------
        def emit_load(sc):
            f4 = ft_pool.tile([128, 4, 128], f32, tag="f4")
            nc.sync.dma_start(out=f4, in_=fe_r[sc])
            ftp = ps_ft.tile([128, SC_ROWS], f32, tag="ftp")
            for q in range(4):
                nc.tensor.transpose(ftp[:, q * 128:(q + 1) * 128], f4[:, q],
                                    ident)
            featT = ft_pool.tile([128, SC_ROWS], f32r, tag="featT")
            nc.scalar.copy(featT, ftp)
            p4 = p_pool.tile([128, 4, D], f32)
            nc.sync.dma_start(out=p4, in_=pr_r[sc])
            stats = st_pool.tile([128, 4, 2, 6], f16)
            return dict(featT=featT, p4=p4, stats=stats, xss=[], bn_insts=[])

        def emit_gemm(st, s):
            xp = ps_x.tile([128, SC_ROWS], f32)
            nc.tensor.matmul(xp, WT[:, s * 128:(s + 1) * 128], st["featT"])
            st.setdefault("xps", []).append(xp)

        def emit_copy_bn(st, s):
            # xs chunk-contiguous fp16; bn_stats reads each chunk-pair with an
            # interleaved AP (i-major, chunk-inner) so the even/odd stat sets
            # are exactly the per-chunk stats.  InstBNStats is built directly
            # because bass's bn_stats() treats 3D inputs as segmented.
            xs = xs_pool.tile([128, 4, VBS], f16)
            nc.scalar.copy(xs, st["xps"][s])
            for j in range(2):
                in3d = xs[:, 2 * j:2 * j + 2, :].transpose([0, 2, 1])
                bi = nc.vector.add_instruction(
                    mybir.InstBNStats(
                        name=nc.vector.bass.get_next_instruction_name(),
                        ins=[nc.vector.lower_ap(in3d)],
                        outs=[nc.vector.lower_ap(st["stats"][:, s, j])]))
                st["bn_insts"].append(bi)
            st["xss"].append(xs)

        def emit_stats(st):
            # stats[p, s, j, (g h)]: g = chunk parity within pair j, h = field
            # (count, mean, M2); chunk c = 2j+g. Per-chunk mean/var direct.
            stats6 = st["stats"].rearrange("p s j (g h) -> p s j g h", g=2)
            mv = stats6[:, :, :, :, 1]
            M2v = stats6[:, :, :, :, 2]
            bn_insts = st["bn_insts"]
            sd = sm_pool.tile([128, 4, 4], f32, tag="sd")
            isd = sm_pool.tile([128, 4, 4], f32, tag="isd")
            sscale = sm_pool.tile([128, 4, 4], f32, tag="sscale")
            tsh = sm_pool.tile([128, 4, 4], f32, tag="tsh")
            tshift = sm_pool.tile([128, 4, 4], f32, tag="tshift")
            i1 = nc.scalar.activation(sd, M2v, AF.Sqrt, bias=eps_t,
                                      scale=1.0 / VBS)
            add_dep_helper(i1.ins, bn_insts[-1].ins, sync=True, reason="bn")
            for bi in bn_insts:
                add_dep_helper(i1.ins, bi.ins, sync=True, reason="stats raw")
            nc.vector.reciprocal(isd, sd)
            nc.gpsimd.tensor_tensor(sscale, isd, gamma44, OP.mult)
            i2 = nc.vector.tensor_tensor(tsh, mv, sscale, OP.mult)
            for bi in bn_insts:
                add_dep_helper(i2.ins, bi.ins, sync=True, reason="stats raw")
            nc.gpsimd.tensor_tensor(tshift, beta44, tsh, OP.subtract)
            st["sscale"] = sscale
            st["tshift"] = tshift

        def emit_chunk_pre(st, c):
            sscale = st["sscale"]
            tshift = st["tshift"]
            xnc = xn_pool.tile([128, 4, VBS], f16, tag="xnc")
            for s in range(4):
                sc_ap = sscale[:, s, c:c + 1]
                sh_ap = tshift[:, s, c:c + 1]
                src = st["xss"][s][:, c]
                dst = xnc[:, s]
                if s < 3:
                    nc.scalar.activation(out=dst, in_=src, func=AF.Identity,
                                         bias=sh_ap, scale=sc_ap)
                else:
                    nc.vector.tensor_scalar(dst, src, sc_ap, sh_ap,
                                            OP.mult, OP.add)
            j = c // 2
            if c % 2 == 0:
                st["ztp2"] = ps_zt.tile([128, 2, D], f16, name="ztp2")
            ztp2 = st["ztp2"]
            for s in range(4):
                nc.tensor.transpose(ztp2[:, c % 2, s * 128:(s + 1) * 128],
                                    xnc[:, s], ident_h)
            if c % 2 == 1:
                z2 = z_pool.tile([128, 2, D], f16, tag="z")
                nc.vector.tensor_tensor(z2, ztp2, st["p4"][:, 2 * j:2 * j + 2],
                                        OP.mult)
                t16 = st["t16q"]
                for cc in (2 * j, 2 * j + 1):
                    z = z2[:, cc % 2]
                    st["z"][cc] = z
                    l1 = sm_pool.tile([128, 16], f16, tag="l1")
                    for h in range(2):
                        nc.vector.max(out=l1[:, h * 8:(h + 1) * 8],
                                      in_=z[:, h * 256:(h + 1) * 256])
                    nc.vector.max(out=t16[:, cc, 0:8], in_=l1)
                    l1m = sm_pool.tile([128, 16], f16, tag="l1m")
                    nc.vector.match_replace(l1m, t16[:, cc, 0:8], l1, NEG)
                    nc.vector.max(out=t16[:, cc, 8:16], in_=l1m)

        def emit_sc_tail(st, sci):
            t16 = st["t16q"]
            # one flat 64-wide cumsum, then per-chunk fixup: for chunk c,
            # cs_c[i] = fy[16c+i] - (fy[16c-1] + 1), with fy[-1] := -1.
            fy = sm_pool.tile([128, 64], f32, tag="fy")
            t16ff = t16.rearrange("p c i -> p (c i)")
            nc.vector.tensor_tensor_scan(fy, t16ff, t16ff,
                                         -1.0, OP.add, OP.bypass)
            offsp = sm_pool.tile([128, 4, 1], f32, tag="offsp")
            nc.gpsimd.memset(offsp[:, 0:1], 0.0)
            nc.vector.tensor_scalar(offsp[:, 1:4],
                                    fy.rearrange("p (c i) -> p c i", i=16)
                                    [:, 0:3, 15:16], 1.0, None, OP.add)
            cs = sm_pool.tile([128, 4, 16], f32, tag="cs")
            nc.vector.tensor_tensor(
                cs, fy.rearrange("p (c i) -> p c i", i=16),
                offsp.broadcast_to([128, 4, 16]),
                OP.subtract)
            rz = sm_pool.tile([128, 4, 16], f32, tag="rz")
            nc.gpsimd.tensor_tensor(rz, t16, rho16q, OP.mult)
            cond = sm_pool.tile([128, 4, 17], f32, tag="cond")
            nc.gpsimd.memset(cond[:, :, 16:17], 0.0)
            nc.vector.tensor_tensor(cond[:, :, 0:16], rz, cs, OP.is_gt)
            dcn = sm_pool.tile([128, 4, 16], f32, tag="dcn")
            nc.gpsimd.tensor_tensor(dcn, cond[:, :, 1:17], cond[:, :, 0:16],
                                    OP.subtract)
            tauj = sm_pool.tile([128, 4, 16], f32, tag="tauj")
            nc.gpsimd.tensor_tensor(tauj, cs, invrhoq, OP.mult)
            scr = sm_pool.tile([128, 4, 16], f32, tag="scr")
            negtau = sm_pool.tile([128, 4], f32, tag="negtau")
            nc.gpsimd.tensor_tensor(scr, tauj, dcn, OP.mult)
            nc.vector.tensor_reduce(out=negtau, in_=scr,
                                    axis=mybir.AxisListType.X, op=OP.add)
            for c in range(4):
                nc.scalar.activation(st["o4"][:, c], st["z"][c], AF.Relu,
                                     bias=negtau[:, c:c + 1], scale=1.0)
            nc.sync.dma_start(out=out_r[sci], in_=st["o4"])

        pend = None
        for sc in range(N_SC):
            st = emit_load(sc)
            if pend is not None:
                pend["o4"] = o_pool.tile([128, 4, D], f32, name="o4", tag="o4")
                pend["z"] = {}
                pend["t16q"] = sm_pool.tile([128, 4, 16], f16, name="t16q",
                                            tag="t16q")
            for i in range(4):
                if pend is not None:
                    emit_chunk_pre(pend, i)
                emit_gemm(st, i)
                emit_copy_bn(st, i)
            if pend is not None:
                emit_sc_tail(pend, sc - 1)
            emit_stats(st)
            pend = st
        pend["o4"] = o_pool.tile([128, 4, D], f32, name="o4", tag="o4")
        pend["z"] = {}
        pend["t16q"] = sm_pool.tile([128, 4, 16], f16, name="t16q", tag="t16q")
        for i in range(4):
            emit_chunk_pre(pend, i)
        emit_sc_tail(pend, N_SC - 1)

    nc.compile()
    return nc


_NC_CACHE = None


def kernel(**inputs) -> np.ndarray:
    global _NC_CACHE
    if _NC_CACHE is None:
        _NC_CACHE = build_nc()
    nc = _NC_CACHE

    priors = np.ascontiguousarray(inputs["priors"], dtype=np.float32)
    feat = np.ascontiguousarray(inputs["processed_feat"], dtype=np.float32)
    W = np.ascontiguousarray(inputs["W"], dtype=np.float32)
    gamma = np.ascontiguousarray(inputs["gamma"], dtype=np.float32)
    beta = np.ascontiguousarray(inputs["beta"], dtype=np.float32)

    in_maps = []
    for i in range(N_CORES):
        sl = slice(i * ROWS, (i + 1) * ROWS)
        in_maps.append({
            "priors": priors[sl],
            "processed_feat": feat[sl],
            "W": W,
            "gamma": gamma,
            "beta": beta,
        })
    res = run_bass_kernel_spmd(nc, in_maps, core_ids=list(range(N_CORES)))
    return np.concatenate([r["out"] for r in res.results], axis=0)


if __name__ == "__main__":
    rng = np.random.default_rng(0)
    inputs = {
        "priors": rng.random((B, D), dtype=np.float32),
        "processed_feat": rng.standard_normal((B, IN), dtype=np.float32),
        "W": (rng.standard_normal((D, IN), dtype=np.float32) * 0.1),
        "gamma": np.ones(D, dtype=np.float32),
        "beta": np.zeros(D, dtype=np.float32),
    }
    out = kernel(**inputs)
    print("out", out.shape, out.dtype, float(out.sum()))
